# revision 1
# baseline (speedup 1.0000x reference)
"""YOLOv5 Detect head (conv 1x1 + sigmoid decode) on 8 Trainium2 NeuronCores.

Data-parallel over batch: core i handles batches [2i, 2i+1].

Per (batch, level) the work is h = W @ x  (W [255, C], x [C, ny*nx]) followed
by the YOLO decode.  On device we compute psum[s, o] = sum_c x[c, s] * wT[c, o]
with the *data* as the stationary operand (lhsT = x tile [K=128, M<=128 spatial])
and wT [K=128, 256] as the moving operand, so the matmul output lands directly
in [spatial, output-channel] orientation: output rows (a*ny*nx + s) are then
contiguous DMA writes, no transpose needed anywhere.

Decode on-chip:
  s = sigmoid(h)                                   (ACT, psum -> sbuf)
  xy cols (o in {0,1}):  2*stride*s + (grid-0.5)*stride   (DVE scalar_tensor_tensor)
  wh cols (o in {2,3}):  (s*s) * (4*anchor)               (DVE tensor_tensor x2)
  rest: s
"""

import numpy as np
from contextlib import ExitStack

import concourse.bacc as bacc
import concourse.bass as bass
import concourse.mybir as mybir
import concourse.tile as tile
from concourse.bass_utils import run_bass_kernel_spmd

F32 = mybir.dt.float32
F32R = mybir.dt.float32r
BF16 = mybir.dt.bfloat16
F16 = mybir.dt.float16
AF = mybir.ActivationFunctionType
OP = mybir.AluOpType

NA, NO = 3, 85
B_TOTAL, N_CORES, B_LOC = 16, 8, 2
RHS_W = NA * NO + 1  # 256: pad 255 -> 256 (fp32r full-rate needs moving dim >= 256)
GRP = 8              # slots (128 spatial rows each) per psum/staging group
ROWS_PER_B = 25200

LEVELS = [
    dict(C=256, nx=80, ny=80, stride=8.0,
         anchors=((10.0, 13.0), (16.0, 30.0), (33.0, 23.0)), base=0),
    dict(C=512, nx=40, ny=40, stride=16.0,
         anchors=((30.0, 61.0), (62.0, 45.0), (59.0, 119.0)), base=19200),
    dict(C=1024, nx=20, ny=20, stride=32.0,
         anchors=((116.0, 90.0), (156.0, 198.0), (373.0, 326.0)), base=24000),
]
for _L in LEVELS:
    _L["S"] = _L["nx"] * _L["ny"]
    _L["KT"] = _L["C"] // 128
    _L["nslots"] = (_L["S"] + 127) // 128
_SB = 0
for _L in LEVELS:
    _L["slot_base"] = _SB
    _SB += _L["nslots"]
TOT_SLOTS = _SB  # 67


def _groups(S):
    """Yield (slot0, n_slots_in_group, rows_in_last_slot)."""
    full, rem = divmod(S, 128)
    gs = [[t0, min(GRP, full - t0), 128] for t0 in range(0, full, GRP)]
    if rem:
        if gs and gs[-1][1] < GRP:
            gs[-1][1] += 1
            gs[-1][2] = rem
        else:
            gs.append([full, 1, rem])
    return [tuple(g) for g in gs]


def _build_program(has_bias: bool, repeat: int = 1, stages: str = "imavo",
                   in_dt: str = "f32r", out_dt: str = "f32"):
    nc = bacc.Bacc("TRN2", target_bir_lowering=False, debug=False,
                   num_devices=N_CORES)

    XDT = {"f32r": F32R, "bf16": BF16, "f16": F16}[in_dt]
    ODT = F32 if out_dt == "f32" else F16
    CDT = ODT  # grid/anchor consts match staging dtype for DVE ops
    xs = [nc.dram_tensor(f"x{l}", [B_LOC, L["C"], L["S"]], XDT,
                         kind="ExternalInput") for l, L in enumerate(LEVELS)]
    wts = [nc.dram_tensor(f"wt{l}", [L["C"], RHS_W], XDT,
                          kind="ExternalInput") for l, L in enumerate(LEVELS)]
    gxs = [nc.dram_tensor(f"gx{l}", [128, L["nslots"]], CDT,
                          kind="ExternalInput") for l, L in enumerate(LEVELS)]
    gys = [nc.dram_tensor(f"gy{l}", [128, L["nslots"]], CDT,
                          kind="ExternalInput") for l, L in enumerate(LEVELS)]
    acs = [nc.dram_tensor(f"ac{l}", [128, NA * 2], CDT,
                          kind="ExternalInput") for l, L in enumerate(LEVELS)]
    if has_bias:
        bts = [nc.dram_tensor(f"bt{l}", [1, RHS_W], F32,
                              kind="ExternalInput") for l, L in enumerate(LEVELS)]
    # slot-major layout: device dumps staging tiles linearly (contiguous
    # ~1MB writes); host reassembles to [16, 25200, 85].
    out_t = nc.dram_tensor("out", [B_LOC, TOT_SLOTS, 128, RHS_W], ODT,
                           kind="ExternalOutput")

    with tile.TileContext(nc) as tc, ExitStack() as ctx:
        cpool = ctx.enter_context(tc.tile_pool(name="consts", bufs=1))
        xbufs = 4 if in_dt in ("bf16", "f16") else 2
        xpools = [ctx.enter_context(tc.tile_pool(name=f"x{l}", bufs=xbufs))
                  for l in range(3)]
        ppool = ctx.enter_context(tc.tile_pool(name="ps", bufs=2, space="PSUM"))
        spool = ctx.enter_context(tc.tile_pool(name="st", bufs=4))
        tpool = ctx.enter_context(tc.tile_pool(name="tmp", bufs=3))

        # --- resident constants ---
        wt_tiles, gx_tiles, gy_tiles, ac_tiles, bt_tiles = [], [], [], [], []
        for l, L in enumerate(LEVELS):
            KT = L["KT"]
            wt = cpool.tile([128, KT * RHS_W], XDT, tag=f"wt{l}")
            nc.sync.dma_start(
                wt[:].rearrange("p (k c) -> p k c", c=RHS_W),
                wts[l][:].rearrange("(k p) c -> p k c", p=128))
            wt_tiles.append(wt)
            gx = cpool.tile([128, L["nslots"]], CDT, tag=f"gx{l}")
            nc.sync.dma_start(gx[:], gxs[l][:])
            gx_tiles.append(gx)
            gy = cpool.tile([128, L["nslots"]], CDT, tag=f"gy{l}")
            nc.sync.dma_start(gy[:], gys[l][:])
            gy_tiles.append(gy)
            ac = cpool.tile([128, NA * 2], CDT, tag=f"ac{l}")
            nc.sync.dma_start(ac[:], acs[l][:])
            ac_tiles.append(ac)
            if has_bias:
                bt = cpool.tile([1, RHS_W], F32, tag=f"bt{l}")
                nc.sync.dma_start(bt[:], bts[l][:])
                bt_tiles.append(bt)
        if has_bias:
            ones = cpool.tile([1, 128], F32, tag="ones")
            nc.vector.memset(ones[:], 1.0)

        # --- main loop ---
        def _emit_body():
          for b in range(B_LOC):
            for l, L in enumerate(LEVELS):
                KT, S = L["KT"], L["S"]
                x_v = xs[l][b].rearrange("(k p) s -> p k s", p=128)
                for (t0, G, M) in _groups(S):
                    s0 = t0 * 128
                    width = (G - 1) * 128 + M  # real spatial columns
                    wfull = G * 128
                    P = 128

                    xt = xpools[l].tile([128, KT * wfull], XDT, tag=f"x{l}")
                    xt_v = xt[:].rearrange("p (k s) -> p k s", s=wfull)
                    if "i" in stages:
                        nc.sync.dma_start(xt_v[:, :, 0:width],
                                          x_v[:, :, s0:s0 + width])
                        if width < wfull:
                            # walrus rejects 16-bit memset; zero via u32 view
                            nc.vector.memset(
                                xt_v[:, :, width:wfull].bitcast(mybir.dt.uint32),
                                0)
                    if "m" not in stages:
                        continue
                    ps = ppool.tile([128, GRP * RHS_W], F32, tag="ps")
                    for j in range(G):
                        po = ps[:, j * RHS_W:(j + 1) * RHS_W]
                        for k in range(KT):
                            nc.tensor.matmul(
                                po,
                                lhsT=xt_v[:, k, j * 128:(j + 1) * 128],
                                rhs=wt_tiles[l][:].rearrange(
                                    "p (k c) -> p k c", c=RHS_W)[:, k, :],
                                start=(k == 0),
                                stop=(k == KT - 1 and not has_bias))
                        if has_bias:
                            nc.tensor.matmul(po, lhsT=ones[0:1, :],
                                             rhs=bt_tiles[l][0:1, :],
                                             start=False, stop=True)

                    if "a" not in stages:
                        continue
                    st = spool.tile([128, GRP * RHS_W], ODT, tag="st")
                    W = G * RHS_W
                    nc.scalar.activation(st[0:P, 0:W], ps[0:P, 0:W], AF.Sigmoid)

                    # decode
                    stv = st[0:P, 0:W].rearrange("p (g w) -> p g w", w=RHS_W)
                    if "v" not in stages:
                        pass
                    else:
                        dat = stv[:, :, 0:NA * NO].rearrange(
                            "p g (a o) -> p g a o", o=NO)
                        xsl = dat[:, :, :, 0]
                        ysl = dat[:, :, :, 1]
                        whs = dat[:, :, :, 2:4]
                        gxb = gx_tiles[l][0:P, t0:t0 + G].unsqueeze(2) \
                            .broadcast_to((P, G, NA))
                        gyb = gy_tiles[l][0:P, t0:t0 + G].unsqueeze(2) \
                            .broadcast_to((P, G, NA))
                        two_sigma = 2.0 * L["stride"]
                        nc.vector.scalar_tensor_tensor(
                            xsl, xsl, two_sigma, gxb, OP.mult, OP.add)
                        nc.vector.scalar_tensor_tensor(
                            ysl, ysl, two_sigma, gyb, OP.mult, OP.add)
                        tmp = tpool.tile([128, GRP * NA * 2], ODT, tag="tmp")
                        tv = tmp[0:P, 0:G * NA * 2].rearrange(
                            "p (g a j) -> p g a j", a=NA, j=2)
                        nc.vector.tensor_tensor(tv, whs, whs, OP.mult)
                        acb = ac_tiles[l][0:P, :].rearrange(
                            "p (a j) -> p a j", j=2).unsqueeze(1) \
                            .broadcast_to((P, G, NA, 2))
                        nc.vector.tensor_tensor(whs, tv, acb, OP.mult)

                    if "o" not in stages:
                        continue
                    sbase = L["slot_base"]
                    dr = out_t[b, sbase + t0:sbase + t0 + G]  # [G, 128, 256]
                    dr_v = dr.rearrange("g p w -> p g w")
                    nc.sync.dma_start(dr_v, stv)

        if repeat == 1:
            _emit_body()
        else:
            # timing-only mode: run the same body `repeat` times via a
            # hardware loop (program size stays constant)
            with tc.For_i(0, repeat, 1,
                          hint_engines=(mybir.EngineType.PE,)):
                _emit_body()

    nc.compile()
    return nc


_PROG_CACHE = {}


def _get_program(has_bias: bool, repeat: int = 1, stages: str = "imavo",
                 in_dt: str = "f32r", out_dt: str = "f32"):
    key = (has_bias, repeat, stages, in_dt, out_dt)
    if key not in _PROG_CACHE:
        _PROG_CACHE[key] = _build_program(has_bias, repeat, stages, in_dt,
                                          out_dt)
    return _PROG_CACHE[key]


def _host_consts(w0, w1, w2, b0, b1, b2, has_bias, in_dt="f32r",
                 out_dt="f32"):
    """Precompute replicated constant arrays shared by all cores."""
    import ml_dtypes
    xdt = {"f32r": np.float32, "bf16": ml_dtypes.bfloat16,
           "f16": np.float16}[in_dt]
    cdt = np.float32 if out_dt == "f32" else np.float16
    consts = {}
    ws, bs = (w0, w1, w2), (b0, b1, b2)
    for l, L in enumerate(LEVELS):
        wT = np.zeros((L["C"], RHS_W), dtype=np.float32)
        wT[:, :NA * NO] = ws[l].T
        consts[f"wt{l}"] = wT.astype(xdt)

        nslots, nx, stride, S = L["nslots"], L["nx"], L["stride"], L["S"]
        s = np.arange(nslots * 128)
        valid = s < S
        gx = np.where(valid, (s % nx - 0.5) * stride, 0.0).astype(np.float32)
        gy = np.where(valid, (s // nx - 0.5) * stride, 0.0).astype(np.float32)
        # gx[p, t] for s = t*128 + p
        consts[f"gx{l}"] = np.ascontiguousarray(
            gx.reshape(nslots, 128).T).astype(cdt)
        consts[f"gy{l}"] = np.ascontiguousarray(
            gy.reshape(nslots, 128).T).astype(cdt)

        ac = (4.0 * np.asarray(L["anchors"], dtype=np.float32)).reshape(1, -1)
        consts[f"ac{l}"] = np.ascontiguousarray(
            np.broadcast_to(ac, (128, NA * 2))).astype(cdt)
        if has_bias:
            bt = np.zeros((1, RHS_W), dtype=np.float32)
            bt[0, :NA * NO] = bs[l]
            consts[f"bt{l}"] = bt
    return consts


def _make_in_maps(inputs, in_dt="f32r", out_dt="f32"):
    x0 = np.asarray(inputs["x0"], dtype=np.float32)
    x1 = np.asarray(inputs["x1"], dtype=np.float32)
    x2 = np.asarray(inputs["x2"], dtype=np.float32)
    w0 = np.asarray(inputs["w0"], dtype=np.float32)
    w1 = np.asarray(inputs["w1"], dtype=np.float32)
    w2 = np.asarray(inputs["w2"], dtype=np.float32)
    b0 = np.asarray(inputs["b0"], dtype=np.float32)
    b1 = np.asarray(inputs["b1"], dtype=np.float32)
    b2 = np.asarray(inputs["b2"], dtype=np.float32)

    has_bias = bool(np.any(b0) or np.any(b1) or np.any(b2))
    consts = _host_consts(w0, w1, w2, b0, b1, b2, has_bias, in_dt, out_dt)

    xr = [x0.reshape(B_TOTAL, LEVELS[0]["C"], LEVELS[0]["S"]),
          x1.reshape(B_TOTAL, LEVELS[1]["C"], LEVELS[1]["S"]),
          x2.reshape(B_TOTAL, LEVELS[2]["C"], LEVELS[2]["S"])]
    if in_dt == "bf16":
        import ml_dtypes
        xr = [a.astype(ml_dtypes.bfloat16) for a in xr]
    elif in_dt == "f16":
        xr = [a.astype(np.float16) for a in xr]

    in_maps = []
    for i in range(N_CORES):
        m = dict(consts)
        for l in range(3):
            m[f"x{l}"] = xr[l][B_LOC * i:B_LOC * (i + 1)]
        in_maps.append(m)
    return in_maps, has_bias


def _assemble_core(raw, dst):
    """raw [B_LOC, TOT_SLOTS, 128, RHS_W] -> dst [B_LOC, 25200, 85]."""
    raw = raw.reshape(B_LOC, TOT_SLOTS, 128, RHS_W)
    if raw.dtype != np.float32:
        raw = raw.astype(np.float32)
    for L in LEVELS:
        S, nslots, sbase = L["S"], L["nslots"], L["slot_base"]
        seg = raw[:, sbase:sbase + nslots].reshape(B_LOC, nslots * 128, RHS_W)
        seg = seg[:, :S, :NA * NO].reshape(B_LOC, S, NA, NO)
        d = dst[:, L["base"]:L["base"] + NA * S].reshape(B_LOC, NA, S, NO)
        d[:] = seg.transpose(0, 2, 1, 3)


def _assemble(results):
    out = np.empty((B_TOTAL, ROWS_PER_B, NO), dtype=np.float32)
    for i in range(N_CORES):
        _assemble_core(results[i]["out"], out[B_LOC * i:B_LOC * (i + 1)])
    return out


IN_DT = "f16"
OUT_DT = "f16"


def _run(inputs, trace=False):
    in_maps, has_bias = _make_in_maps(inputs, IN_DT, OUT_DT)
    nc = _get_program(has_bias, in_dt=IN_DT, out_dt=OUT_DT)
    res = run_bass_kernel_spmd(nc, in_maps, core_ids=list(range(N_CORES)),
                               trace=trace)
    return _assemble(res.results), res


def kernel(**inputs):
    out, _ = _run(inputs, trace=False)
    return out



# revision 2
# speedup vs baseline: 1.0790x; 1.0790x over previous
"""YOLOv5 Detect head (conv 1x1 + sigmoid decode) on 8 Trainium2 NeuronCores.

Data-parallel over batch: core i handles batches [2i, 2i+1].

Per (batch, level) the work is h = W @ x  (W [255, C], x [C, ny*nx]) followed
by the YOLO decode.  On device we compute psum[s, o] = sum_c x[c, s] * wT[c, o]
with the *data* as the stationary operand (lhsT = x tile [K=128, M=128 spatial])
and wT [K=128, 256] as the moving operand, so the matmul output lands directly
in [spatial, output-channel] orientation.

Optimizations over the f16 baseline:
  - fp8 (e4m3) x and W, each pre-scaled by 16 on host (avoids subnormal
    flush for the small weights); the sigmoid's free input scale (1/256)
    undoes it.  Halves input DMA bytes; matmul runs at full rate via FWL.
  - W columns permuted: cols 0..11 = xywh channels (x,y per anchor then
    w,h per anchor), cols 12..254 = obj/cls scores, col 255 = zero pad.
  - Scores (95% of output bytes): per-group ACT sigmoid PSUM -> fp8 tile
    -> DMA with per-partition-contiguous G*243B runs.
  - xywh: DVE copies pre-sigmoid psum cols 0:12 (f16) into a per-batch
    tile; one batch-end sigmoid + 3 DVE ops decode all 67 slots at once
    (per-slot constants M1/G6/anchors are baked host-side), one DMA.
"""

import numpy as np
from contextlib import ExitStack

import concourse.bacc as bacc
import concourse.bass as bass
import concourse.mybir as mybir
import concourse.tile as tile
from concourse.bass_utils import run_bass_kernel_spmd

F32 = mybir.dt.float32
F16 = mybir.dt.float16
F8 = mybir.dt.float8e4
AF = mybir.ActivationFunctionType
OP = mybir.AluOpType

NA, NO = 3, 85
B_TOTAL, N_CORES, B_LOC = 16, 8, 2
RHS_W = 256          # 12 xywh + 243 scores + 1 pad
XY_W = 12            # decoded channels per spatial row
SC_W = NA * (NO - 4)  # 243 score channels
GRP = 8              # slots (128 spatial rows each) per psum group
ROWS_PER_B = 25200
W_SCALE = 16.0       # host-side scale on each of x and w; ACT undoes 1/256

LEVELS = [
    dict(C=256, nx=80, ny=80, stride=8.0,
         anchors=((10.0, 13.0), (16.0, 30.0), (33.0, 23.0)), base=0),
    dict(C=512, nx=40, ny=40, stride=16.0,
         anchors=((30.0, 61.0), (62.0, 45.0), (59.0, 119.0)), base=19200),
    dict(C=1024, nx=20, ny=20, stride=32.0,
         anchors=((116.0, 90.0), (156.0, 198.0), (373.0, 326.0)), base=24000),
]
for _L in LEVELS:
    _L["S"] = _L["nx"] * _L["ny"]
    _L["KT"] = _L["C"] // 128
    _L["nslots"] = (_L["S"] + 127) // 128
_SB = 0
for _L in LEVELS:
    _L["slot_base"] = _SB
    _SB += _L["nslots"]
TOT_SLOTS = _SB  # 67


def _perm():
    """perm[new_col] = original output-channel row of W (0..254)."""
    p = np.empty(NA * NO, dtype=np.int64)
    for j in range(6):
        p[j] = (j // 2) * NO + (j % 2)          # x,y per anchor
    for j in range(6):
        p[6 + j] = (j // 2) * NO + 2 + (j % 2)  # w,h per anchor
    for a in range(NA):
        for i in range(NO - 4):
            p[XY_W + a * (NO - 4) + i] = a * NO + 4 + i
    return p


def _groups(S):
    """Yield (slot0, n_slots_in_group, rows_in_last_slot)."""
    full, rem = divmod(S, 128)
    gs = [[t0, min(GRP, full - t0), 128] for t0 in range(0, full, GRP)]
    if rem:
        if gs and gs[-1][1] < GRP:
            gs[-1][1] += 1
            gs[-1][2] = rem
        else:
            gs.append([full, 1, rem])
    return [tuple(g) for g in gs]


def _build_program(has_bias: bool, repeat: int = 1, stages: str = "imavo",
                   in_dt: str = "f8", out_dt: str = "f8"):
    nc = bacc.Bacc("TRN2", target_bir_lowering=False, debug=False,
                   num_devices=N_CORES)

    xs = [nc.dram_tensor(f"x{l}", [B_LOC, L["C"], L["S"]], F8,
                         kind="ExternalInput") for l, L in enumerate(LEVELS)]
    wts = [nc.dram_tensor(f"wt{l}", [L["C"], RHS_W], F8,
                          kind="ExternalInput") for l, L in enumerate(LEVELS)]
    m1_d = nc.dram_tensor("m1", [128, TOT_SLOTS * XY_W], F16,
                          kind="ExternalInput")
    g6_d = nc.dram_tensor("g6", [128, TOT_SLOTS * 6], F16,
                          kind="ExternalInput")
    if has_bias:
        bts = [nc.dram_tensor(f"bt{l}", [1, RHS_W], F32,
                              kind="ExternalInput") for l, L in enumerate(LEVELS)]
    # outputs: per-partition-contiguous layout [b, p, slot, ch]
    osc = nc.dram_tensor("osc", [B_LOC, 128, TOT_SLOTS, SC_W], F8,
                         kind="ExternalOutput")
    oxy = nc.dram_tensor("oxy", [B_LOC, 128, TOT_SLOTS, XY_W], F16,
                         kind="ExternalOutput")

    with tile.TileContext(nc) as tc, ExitStack() as ctx:
        cpool = ctx.enter_context(tc.tile_pool(name="consts", bufs=1))
        xpools = [ctx.enter_context(tc.tile_pool(name=f"x{l}", bufs=2))
                  for l in range(3)]
        ppool = ctx.enter_context(tc.tile_pool(name="ps", bufs=2, space="PSUM"))
        scpool = ctx.enter_context(tc.tile_pool(name="sc", bufs=4))
        hpool = ctx.enter_context(tc.tile_pool(name="h12", bufs=2))
        dpool = ctx.enter_context(tc.tile_pool(name="dec", bufs=2))

        # --- resident constants ---
        wt_tiles, bt_tiles = [], []
        for l, L in enumerate(LEVELS):
            KT = L["KT"]
            wt = cpool.tile([128, KT * RHS_W], F8, tag=f"wt{l}")
            nc.sync.dma_start(
                wt[:].rearrange("p (k c) -> p k c", c=RHS_W),
                wts[l][:].rearrange("(k p) c -> p k c", p=128))
            wt_tiles.append(wt)
            if has_bias:
                bt = cpool.tile([1, RHS_W], F32, tag=f"bt{l}")
                nc.sync.dma_start(bt[:], bts[l][:])
                bt_tiles.append(bt)
        m1_t = cpool.tile([128, TOT_SLOTS * XY_W], F16, tag="m1")
        nc.sync.dma_start(m1_t[:], m1_d[:])
        g6_t = cpool.tile([128, TOT_SLOTS * 6], F16, tag="g6")
        nc.sync.dma_start(g6_t[:], g6_d[:])
        if has_bias:
            ones = cpool.tile([1, 128], F32, tag="ones")
            nc.vector.memset(ones[:], 1.0)

        inv = 1.0 / (W_SCALE * W_SCALE)

        # --- main loop ---
        def _emit_body():
          for b in range(B_LOC):
            h12 = hpool.tile([128, TOT_SLOTS * XY_W], F16, tag="h12")
            h12_v = h12[:].rearrange("p (t j) -> p t j", j=XY_W)
            for l, L in enumerate(LEVELS):
                KT, S, sbase = L["KT"], L["S"], L["slot_base"]
                x_v = xs[l][b].rearrange("(k p) s -> p k s", p=128)
                for (t0, G, M) in _groups(S):
                    s0 = t0 * 128
                    width = (G - 1) * 128 + M  # real spatial columns
                    wfull = G * 128

                    xt = xpools[l].tile([128, KT * wfull], F8, tag=f"x{l}")
                    xt_v = xt[:].rearrange("p (k s) -> p k s", s=wfull)
                    if "i" in stages:
                        nc.sync.dma_start(xt_v[:, :, 0:width],
                                          x_v[:, :, s0:s0 + width])
                        if width < wfull:
                            nc.vector.memset(
                                xt_v[:, :, width:wfull].bitcast(mybir.dt.uint32),
                                0)
                    if "m" not in stages:
                        continue
                    ps = ppool.tile([128, GRP * RHS_W], F32, tag="ps")
                    ps_v = ps[:].rearrange("p (g w) -> p g w", w=RHS_W)
                    for j in range(G):
                        po = ps[:, j * RHS_W:(j + 1) * RHS_W]
                        for k in range(KT):
                            nc.tensor.matmul(
                                po,
                                lhsT=xt_v[:, k, j * 128:(j + 1) * 128],
                                rhs=wt_tiles[l][:].rearrange(
                                    "p (k c) -> p k c", c=RHS_W)[:, k, :],
                                start=(k == 0),
                                stop=(k == KT - 1 and not has_bias))
                        if has_bias:
                            nc.tensor.matmul(po, lhsT=ones[0:1, :],
                                             rhs=bt_tiles[l][0:1, :],
                                             start=False, stop=True)

                    # scores: sigmoid psum cols 12:255 -> fp8, then DMA out
                    if "a" in stages:
                        sc = scpool.tile([128, GRP * SC_W], F8, tag="sc")
                        sc_v = sc[:].rearrange("p (g c) -> p g c", c=SC_W)
                        nc.scalar.activation(sc_v[:, 0:G, :],
                                             ps_v[:, 0:G, XY_W:XY_W + SC_W],
                                             AF.Sigmoid, scale=inv)
                        if "o" in stages:
                            nc.sync.dma_start(
                                osc[b][:, sbase + t0:sbase + t0 + G, :],
                                sc_v[:, 0:G, :])

                    # xywh: stash pre-sigmoid psum cols 0:12 (f16)
                    if "v" in stages:
                        nc.vector.tensor_copy(
                            h12_v[:, sbase + t0:sbase + t0 + G, :],
                            ps_v[:, 0:G, 0:XY_W])

            if "v" not in stages:
                continue
            # batch-end decode of all 67 slots
            s12 = dpool.tile([128, TOT_SLOTS * XY_W], F16, tag="s12")
            nc.scalar.activation(s12[:], h12[:], AF.Sigmoid, scale=inv)
            t12 = dpool.tile([128, TOT_SLOTS * XY_W], F16, tag="t12")
            # t = sigma * M1   (M1: 2*stride for xy cols, 4*anchor for wh)
            nc.vector.tensor_tensor(t12[:], s12[:], m1_t[:], OP.mult)
            s12_v = s12[:].rearrange("p (t j) -> p t j", j=XY_W)
            t12_v = t12[:].rearrange("p (t j) -> p t j", j=XY_W)
            g6_v = g6_t[:].rearrange("p (t j) -> p t j", j=6)
            # xy = t + (grid - 0.5) * stride
            nc.vector.tensor_tensor(s12_v[:, :, 0:6], t12_v[:, :, 0:6],
                                    g6_v, OP.add)
            # wh = t * sigma = 4 * anchor * sigma^2
            nc.vector.tensor_tensor(s12_v[:, :, 6:XY_W], t12_v[:, :, 6:XY_W],
                                    s12_v[:, :, 6:XY_W], OP.mult)
            if "o" in stages:
                nc.sync.dma_start(oxy[b][:, :, :], s12_v)

        if repeat == 1:
            _emit_body()
        else:
            # timing-only mode: run the same body `repeat` times via a
            # hardware loop (program size stays constant)
            with tc.For_i(0, repeat, 1,
                          hint_engines=(mybir.EngineType.PE,)):
                _emit_body()

    nc.compile()
    return nc


_PROG_CACHE = {}


def _get_program(has_bias: bool, repeat: int = 1, stages: str = "imavo",
                 in_dt: str = "f8", out_dt: str = "f8"):
    key = (has_bias, repeat, stages, in_dt, out_dt)
    if key not in _PROG_CACHE:
        _PROG_CACHE[key] = _build_program(has_bias, repeat, stages, in_dt,
                                          out_dt)
    return _PROG_CACHE[key]


def _host_consts(w0, w1, w2, b0, b1, b2, has_bias):
    """Precompute replicated constant arrays shared by all cores."""
    import ml_dtypes
    f8 = ml_dtypes.float8_e4m3
    perm = _perm()
    consts = {}
    ws, bs = (w0, w1, w2), (b0, b1, b2)
    for l, L in enumerate(LEVELS):
        wT = np.zeros((L["C"], RHS_W), dtype=np.float32)
        wT[:, :NA * NO] = (W_SCALE * ws[l][perm]).T
        consts[f"wt{l}"] = wT.astype(f8)
        if has_bias:
            bt = np.zeros((1, RHS_W), dtype=np.float32)
            bt[0, :NA * NO] = (W_SCALE * W_SCALE) * bs[l][perm]
            consts[f"bt{l}"] = bt

    m1 = np.zeros((128, TOT_SLOTS, XY_W), dtype=np.float32)
    g6 = np.zeros((128, TOT_SLOTS, 6), dtype=np.float32)
    for L in LEVELS:
        sb, nsl, nx, stride, S = (L["slot_base"], L["nslots"], L["nx"],
                                  L["stride"], L["S"])
        m1[:, sb:sb + nsl, 0:6] = 2.0 * stride
        anc = np.asarray(L["anchors"], dtype=np.float32).reshape(6)
        m1[:, sb:sb + nsl, 6:12] = 4.0 * anc[None, None, :]
        s = np.arange(nsl * 128)
        valid = s < S
        gx = np.where(valid, (s % nx - 0.5) * stride, 0.0).astype(np.float32)
        gy = np.where(valid, (s // nx - 0.5) * stride, 0.0).astype(np.float32)
        g6[:, sb:sb + nsl, 0::2] = gx.reshape(nsl, 128).T[:, :, None]
        g6[:, sb:sb + nsl, 1::2] = gy.reshape(nsl, 128).T[:, :, None]
    consts["m1"] = np.ascontiguousarray(
        m1.reshape(128, TOT_SLOTS * XY_W)).astype(np.float16)
    consts["g6"] = np.ascontiguousarray(
        g6.reshape(128, TOT_SLOTS * 6)).astype(np.float16)
    return consts


def _make_in_maps(inputs, in_dt="f8", out_dt="f8"):
    import ml_dtypes
    f8 = ml_dtypes.float8_e4m3
    x0 = np.asarray(inputs["x0"], dtype=np.float32)
    x1 = np.asarray(inputs["x1"], dtype=np.float32)
    x2 = np.asarray(inputs["x2"], dtype=np.float32)
    w0 = np.asarray(inputs["w0"], dtype=np.float32)
    w1 = np.asarray(inputs["w1"], dtype=np.float32)
    w2 = np.asarray(inputs["w2"], dtype=np.float32)
    b0 = np.asarray(inputs["b0"], dtype=np.float32)
    b1 = np.asarray(inputs["b1"], dtype=np.float32)
    b2 = np.asarray(inputs["b2"], dtype=np.float32)

    has_bias = bool(np.any(b0) or np.any(b1) or np.any(b2))
    consts = _host_consts(w0, w1, w2, b0, b1, b2, has_bias)

    xr = [(W_SCALE * x0.reshape(B_TOTAL, LEVELS[0]["C"], LEVELS[0]["S"]))
          .astype(f8),
          (W_SCALE * x1.reshape(B_TOTAL, LEVELS[1]["C"], LEVELS[1]["S"]))
          .astype(f8),
          (W_SCALE * x2.reshape(B_TOTAL, LEVELS[2]["C"], LEVELS[2]["S"]))
          .astype(f8)]

    in_maps = []
    for i in range(N_CORES):
        m = dict(consts)
        for l in range(3):
            m[f"x{l}"] = xr[l][B_LOC * i:B_LOC * (i + 1)]
        in_maps.append(m)
    return in_maps, has_bias


def _assemble_core(res, dst):
    """res osc [B,128,67,243] f8 + oxy [B,128,67,12] f16 -> dst [B,25200,85]."""
    oxy = np.asarray(res["oxy"]).astype(np.float32)
    sc = np.asarray(res["osc"]).astype(np.float32)
    for L in LEVELS:
        S, nsl, sb = L["S"], L["nslots"], L["slot_base"]
        xy = oxy[:, :, sb:sb + nsl, :].transpose(0, 2, 1, 3) \
            .reshape(B_LOC, nsl * 128, XY_W)[:, :S]
        s = sc[:, :, sb:sb + nsl, :].transpose(0, 2, 1, 3) \
            .reshape(B_LOC, nsl * 128, SC_W)[:, :S]
        d = dst[:, L["base"]:L["base"] + NA * S].reshape(B_LOC, NA, S, NO)
        for a in range(NA):
            d[:, a, :, 0] = xy[:, :, 2 * a]
            d[:, a, :, 1] = xy[:, :, 2 * a + 1]
            d[:, a, :, 2] = xy[:, :, 6 + 2 * a]
            d[:, a, :, 3] = xy[:, :, 7 + 2 * a]
            d[:, a, :, 4:] = s[:, :, (NO - 4) * a:(NO - 4) * (a + 1)]


def _assemble(results):
    out = np.empty((B_TOTAL, ROWS_PER_B, NO), dtype=np.float32)
    for i in range(N_CORES):
        _assemble_core(results[i], out[B_LOC * i:B_LOC * (i + 1)])
    return out


IN_DT = "f8"
OUT_DT = "f8"


def _run(inputs, trace=False):
    in_maps, has_bias = _make_in_maps(inputs, IN_DT, OUT_DT)
    nc = _get_program(has_bias, in_dt=IN_DT, out_dt=OUT_DT)
    res = run_bass_kernel_spmd(nc, in_maps, core_ids=list(range(N_CORES)),
                               trace=trace)
    return _assemble(res.results), res


def kernel(**inputs):
    out, _ = _run(inputs, trace=False)
    return out


# revision 23
# speedup vs baseline: 1.1905x; 1.1033x over previous
"""YOLOv5 Detect head (conv 1x1 + sigmoid decode) on 8 Trainium2 NeuronCores.

Data-parallel over batch: core i handles batches [2i, 2i+1].

Per (batch, level) the work is h = W @ x  (W [255, C], x [C, ny*nx]) followed
by the YOLO decode.  On device we compute psum[s, o] = sum_c x[c, s] * wT[c, o]
with the *data* as the stationary operand (lhsT = x tile [K=128, M=128 spatial])
and wT [K=128, 256] as the moving operand, so the matmul output lands directly
in [spatial, output-channel] orientation.

Optimizations over the f16 baseline:
  - fp8 (e4m3) x and W, each pre-scaled by 16 on host (avoids subnormal
    flush for the small weights); the sigmoid's free input scale (1/256)
    undoes it.  Halves input DMA bytes; matmul runs at full rate via FWL.
  - W columns permuted: cols 0..11 = xywh channels (x,y per anchor then
    w,h per anchor), cols 12..254 = obj/cls scores, col 255 = zero pad.
  - Scores (95% of output bytes): per-group ACT sigmoid PSUM -> fp8 tile
    -> DMA with per-partition-contiguous G*243B runs.
  - xywh: DVE copies pre-sigmoid psum cols 0:12 (f16) into a per-batch
    tile; one batch-end sigmoid + 3 DVE ops decode all 67 slots at once
    (per-slot constants M1/G6/anchors are baked host-side), one DMA.
"""

import numpy as np
from contextlib import ExitStack

import concourse.bacc as bacc
import concourse.bass as bass
import concourse.mybir as mybir
import concourse.tile as tile
from concourse.bass_utils import run_bass_kernel_spmd

F32 = mybir.dt.float32
F16 = mybir.dt.float16
F8 = mybir.dt.float8e4
AF = mybir.ActivationFunctionType
OP = mybir.AluOpType

NA, NO = 3, 85
B_TOTAL, N_CORES, B_LOC = 16, 8, 2
RHS_W = 256          # 12 xywh + 243 scores + 1 pad
XY_W = 12            # decoded channels per spatial row
SC_W = NA * (NO - 4)  # 243 score channels
GRP = 8              # slots (128 spatial rows each) per psum group (4 banks)
ROWS_PER_B = 25200
W_SCALE = 16.0       # host-side scale on each of x and w; ACT undoes 1/256

LEVELS = [
    dict(C=256, nx=80, ny=80, stride=8.0,
         anchors=((10.0, 13.0), (16.0, 30.0), (33.0, 23.0)), base=0),
    dict(C=512, nx=40, ny=40, stride=16.0,
         anchors=((30.0, 61.0), (62.0, 45.0), (59.0, 119.0)), base=19200),
    dict(C=1024, nx=20, ny=20, stride=32.0,
         anchors=((116.0, 90.0), (156.0, 198.0), (373.0, 326.0)), base=24000),
]
for _L in LEVELS:
    _L["S"] = _L["nx"] * _L["ny"]
    _L["KT"] = _L["C"] // 128
    _L["nslots"] = (_L["S"] + 127) // 128
_SB = 0
for _L in LEVELS:
    _L["slot_base"] = _SB
    _SB += _L["nslots"]
TOT_SLOTS = _SB  # 67


def _perm():
    """perm[new_col] = original output-channel row of W (0..254)."""
    p = np.empty(NA * NO, dtype=np.int64)
    for j in range(6):
        p[j] = (j // 2) * NO + (j % 2)          # x,y per anchor
    for j in range(6):
        p[6 + j] = (j // 2) * NO + 2 + (j % 2)  # w,h per anchor
    for a in range(NA):
        for i in range(NO - 4):
            p[XY_W + a * (NO - 4) + i] = a * NO + 4 + i
    return p


def _groups(S):
    """Yield (slot0, n_slots_in_group, rows_in_last_slot)."""
    full, rem = divmod(S, 128)
    gs = [[t0, min(GRP, full - t0), 128] for t0 in range(0, full, GRP)]
    if rem:
        if gs and gs[-1][1] < GRP:
            gs[-1][1] += 1
            gs[-1][2] = rem
        else:
            gs.append([full, 1, rem])
    return [tuple(g) for g in gs]


def _build_program(has_bias: bool, repeat: int = 1, stages: str = "imavo",
                   in_dt: str = "f8", out_dt: str = "f8", dup: int = 1):
    nc = bacc.Bacc("TRN2", target_bir_lowering=False, debug=False,
                   num_devices=N_CORES)

    xs = [nc.dram_tensor(f"x{l}", [B_LOC, L["C"], L["S"]], F8,
                         kind="ExternalInput") for l, L in enumerate(LEVELS)]
    wts = [nc.dram_tensor(f"wt{l}", [L["C"], RHS_W], F8,
                          kind="ExternalInput") for l, L in enumerate(LEVELS)]
    m1_d = nc.dram_tensor("m1", [128, TOT_SLOTS * XY_W], F16,
                          kind="ExternalInput")
    g6_d = nc.dram_tensor("g6", [128, TOT_SLOTS * 6], F16,
                          kind="ExternalInput")
    if has_bias:
        bts = [nc.dram_tensor(f"bt{l}", [1, RHS_W], F32,
                              kind="ExternalInput") for l, L in enumerate(LEVELS)]
    # outputs: per-partition-contiguous layout [b, p, slot, ch]
    osc = nc.dram_tensor("osc", [B_LOC, 128, TOT_SLOTS, SC_W], F8,
                         kind="ExternalOutput")
    oxy = nc.dram_tensor("oxy", [B_LOC, 128, TOT_SLOTS, XY_W], F16,
                         kind="ExternalOutput")

    with tile.TileContext(nc) as tc, ExitStack() as ctx:
        cpool = ctx.enter_context(tc.tile_pool(name="consts", bufs=1))
        xpools = [ctx.enter_context(tc.tile_pool(name=f"x{l}", bufs=2))
                  for l in range(3)]
        ppool = ctx.enter_context(tc.tile_pool(name="ps", bufs=2, space="PSUM"))
        scpool = ctx.enter_context(tc.tile_pool(name="sc", bufs=2))
        hpool = ctx.enter_context(tc.tile_pool(name="h12", bufs=2))
        dpool = ctx.enter_context(tc.tile_pool(name="dec", bufs=2))
        hspool = ctx.enter_context(tc.tile_pool(name="hs", bufs=2))

        # --- resident constants (ACT hwdge ring: keep the SP ring free for
        # the x planes so the first matmul isn't queued behind ~0.8MB) ---
        wt_tiles, bt_tiles = [], []
        for l, L in enumerate(LEVELS):
            KT = L["KT"]
            wt = cpool.tile([128, KT * RHS_W], F8, tag=f"wt{l}")
            nc.scalar.dma_start(
                wt[:].rearrange("p (k c) -> p k c", c=RHS_W),
                wts[l][:].rearrange("(k p) c -> p k c", p=128))
            wt_tiles.append(wt)
            if has_bias:
                bt = cpool.tile([1, RHS_W], F32, tag=f"bt{l}")
                nc.scalar.dma_start(bt[:], bts[l][:])
                bt_tiles.append(bt)
        m1_t = cpool.tile([128, TOT_SLOTS * XY_W], F16, tag="m1")
        nc.scalar.dma_start(m1_t[:], m1_d[:])
        g6_t = cpool.tile([128, TOT_SLOTS * 6], F16, tag="g6")
        nc.scalar.dma_start(g6_t[:], g6_d[:])
        if has_bias:
            ones = cpool.tile([1, 128], F32, tag="ones")
            nc.vector.memset(ones[:], 1.0)

        inv = 1.0 / (W_SCALE * W_SCALE)

        # --- main loop ---
        def _emit_decode(b, h12, t_lo, t_hi, tag):
            """Sigmoid + decode slots [t_lo, t_hi) of h12, DMA to oxy."""
            n = t_hi - t_lo
            W = n * XY_W
            s12 = dpool.tile([128, TOT_SLOTS * XY_W], F16, tag=f"s12{tag}")
            nc.scalar.activation(s12[:, 0:W],
                                 h12[:, t_lo * XY_W:t_hi * XY_W],
                                 AF.Sigmoid, scale=inv)
            t12 = dpool.tile([128, TOT_SLOTS * XY_W], F16, tag=f"t12{tag}")
            # t = sigma * M1   (M1: 2*stride for xy cols, 4*anchor for wh)
            nc.vector.tensor_tensor(t12[:, 0:W], s12[:, 0:W],
                                    m1_t[:, t_lo * XY_W:t_hi * XY_W], OP.mult)
            s12_v = s12[:].rearrange("p (t j) -> p t j", j=XY_W)
            t12_v = t12[:].rearrange("p (t j) -> p t j", j=XY_W)
            g6_v = g6_t[:].rearrange("p (t j) -> p t j", j=6)
            # xy = t + (grid - 0.5) * stride
            nc.vector.tensor_tensor(s12_v[:, 0:n, 0:6], t12_v[:, 0:n, 0:6],
                                    g6_v[:, t_lo:t_hi, :], OP.add)
            # wh = t * sigma = 4 * anchor * sigma^2
            nc.vector.tensor_tensor(s12_v[:, 0:n, 6:XY_W],
                                    t12_v[:, 0:n, 6:XY_W],
                                    s12_v[:, 0:n, 6:XY_W], OP.mult)
            if "o" in stages:
                nc.scalar.dma_start(oxy[b][:, t_lo:t_hi, :], s12_v[:, 0:n, :])

        def _emit_body():
          for b in range(B_LOC):
            h12 = hpool.tile([128, TOT_SLOTS * XY_W], F16, tag="h12")
            h12_v = h12[:].rearrange("p (t j) -> p t j", j=XY_W)

            xt_vs, sc_tiles, sc_vs = [], [], []
            n_defer = sum(1 for (_, G, _) in _groups(LEVELS[0]["S"])
                          if G == GRP)
            hs_v = None
            for l, L in enumerate(LEVELS):
                KT, S = L["KT"], L["S"]
                Wl = L["nslots"] * 128  # padded spatial width
                x_v = xs[l][b].rearrange("(k p) s -> p k s", p=128)
                # whole-level x tile; one DMA per (b, level) gives S-byte
                # contiguous runs per partition and minimizes HWDGE load
                xt = xpools[l].tile([128, KT * Wl], F8, tag=f"x{l}")
                xt_v = xt[:].rearrange("p (k s) -> p k s", s=Wl)
                xt_vs.append(xt_v)
                if "i" in stages:
                    if b == 0 and l == 0:
                        # head-split: first matmul group only needs the
                        # first GRP slots, don't make it wait for 1.6MB
                        hd = GRP * 128
                        nc.sync.dma_start(xt_v[:, :, 0:hd],
                                          x_v[:, :, 0:hd])
                        nc.sync.dma_start(xt_v[:, :, hd:S],
                                          x_v[:, :, hd:S])
                    else:
                        nc.sync.dma_start(xt_v[:, :, 0:S], x_v[:, :, :])
                    if S < Wl:
                        nc.vector.memset(
                            xt_v[:, :, S:Wl].bitcast(mybir.dt.uint32), 0)
                if "a" in stages:
                    # per-level fp8 score staging; ACT fills it per psum
                    # group, one big DMA per (b, level) writes it out
                    sc = scpool.tile([128, L["nslots"] * SC_W], F8,
                                     tag=f"sc{l}")
                    sc_tiles.append(sc)
                    sc_vs.append(sc[:].rearrange("p (t c) -> p t c", c=SC_W))
                    if l == 0 and n_defer and "v" in stages:
                        hs = hspool.tile([128, n_defer * 2 * SC_W], F16,
                                         tag="hs")
                        hs_v = hs[:].rearrange("p (g s c) -> p g s c",
                                               s=2, c=SC_W)
            if "m" not in stages:
                continue

            # interleave the ACT-heavy L0 groups with PE-heavy L1/L2 groups
            # so the psum drain chain hides under neighbor compute; L2's
            # group stays last (smallest decode tail)
            l0g = [(0, g) for g in _groups(LEVELS[0]["S"])]
            l1g = [(1, g) for g in _groups(LEVELS[1]["S"])]
            l2g = [(2, g) for g in _groups(LEVELS[2]["S"])]
            order, ri = [], 0
            for i, g in enumerate(l0g):
                order.append(g)
                if i % 2 == 1 and ri < len(l1g):
                    order.append(l1g[ri])
                    ri += 1
            order.extend(l1g[ri:])
            order.extend(l2g)
            remaining = [len(_groups(L["S"])) for L in LEVELS]

            for (l, (t0, G, M)) in order:
                L = LEVELS[l]
                KT, sbase = L["KT"], L["slot_base"]
                xt_v = xt_vs[l]
                ps = ppool.tile([128, GRP * RHS_W], F32, tag="ps")
                ps_v = ps[:].rearrange("p (g w) -> p g w", w=RHS_W)
                for j in range(G):
                    po = ps[:, j * RHS_W:(j + 1) * RHS_W]
                    for k in range(KT):
                        nc.tensor.matmul(
                            po,
                            lhsT=xt_v[:, k,
                                      (t0 + j) * 128:(t0 + j + 1) * 128],
                            rhs=wt_tiles[l][:].rearrange(
                                "p (k c) -> p k c", c=RHS_W)[:, k, :],
                            start=(k == 0),
                            stop=(k == KT - 1 and not has_bias))
                    if has_bias:
                        nc.tensor.matmul(po, lhsT=ones[0:1, :],
                                         rhs=bt_tiles[l][0:1, :],
                                         start=False, stop=True)

                # scores: sigmoid psum -> fp8 staging.  For L0's full
                # groups, drain the last 2 slots via DVE (f16 raw h) and
                # sigmoid them in one deferred batched ACT from SBUF so
                # the per-group ACT stays under the PE group time.
                defer = (l == 0 and G == GRP and "a" in stages
                         and "v" in stages)
                if "a" in stages:
                    sc_v = sc_vs[l]
                    nd = G - 2 if defer else G
                    nc.scalar.activation(sc_v[:, t0:t0 + nd, :],
                                         ps_v[:, 0:nd, XY_W:XY_W + SC_W],
                                         AF.Sigmoid, scale=inv)
                    if defer:
                        nc.vector.tensor_copy(
                            hs_v[:, t0 // GRP, :, :],
                            ps_v[:, G - 2:G, XY_W:XY_W + SC_W])

                # xywh: stash pre-sigmoid psum cols 0:12 (f16)
                if "v" in stages:
                    nc.vector.tensor_copy(
                        h12_v[:, sbase + t0:sbase + t0 + G, :],
                        ps_v[:, 0:G, 0:XY_W])

                remaining[l] -= 1
                if remaining[l] == 0 and "a" in stages:
                    if l == 0 and n_defer and "v" in stages:
                        # deferred sigmoid of the DVE-drained L0 slots
                        sc4 = sc_tiles[0][:, 0:n_defer * GRP * SC_W] \
                            .rearrange("p (g s c) -> p g s c",
                                       s=GRP, c=SC_W)
                        nc.scalar.activation(sc4[:, :, GRP - 2:GRP, :],
                                             hs_v[:, 0:n_defer, :, :],
                                             AF.Sigmoid, scale=inv)
                    if "o" in stages:
                        nc.scalar.dma_start(
                            osc[b][:, sbase:sbase + L["nslots"], :],
                            sc_vs[l])
                # decode early once L0+L1 slots are all stashed; L2's 4
                # slots are the only decode left in the kernel tail
                if ("v" in stages and remaining[0] == 0
                        and remaining[1] == 0 and remaining[2] > 0
                        and l != 2):
                    _emit_decode(b, h12, 0, LEVELS[2]["slot_base"], "a")
            if "v" in stages:
                _emit_decode(b, h12, LEVELS[2]["slot_base"], TOT_SLOTS, "b")

        if repeat == 1:
            _emit_body()
        else:
            # timing-only mode: run the same body `repeat` times via a
            # hardware loop (program size stays constant)
            with tc.For_i(0, repeat, 1,
                          hint_engines=(mybir.EngineType.PE,)):
                for _ in range(dup):
                    _emit_body()

    nc.compile()
    return nc


_PROG_CACHE = {}


def _get_program(has_bias: bool, repeat: int = 1, stages: str = "imavo",
                 in_dt: str = "f8", out_dt: str = "f8", dup: int = 1):
    key = (has_bias, repeat, stages, in_dt, out_dt, dup)
    if key not in _PROG_CACHE:
        _PROG_CACHE[key] = _build_program(has_bias, repeat, stages, in_dt,
                                          out_dt, dup)
    return _PROG_CACHE[key]


def _host_consts(w0, w1, w2, b0, b1, b2, has_bias):
    """Precompute replicated constant arrays shared by all cores."""
    import ml_dtypes
    f8 = ml_dtypes.float8_e4m3
    perm = _perm()
    consts = {}
    ws, bs = (w0, w1, w2), (b0, b1, b2)
    for l, L in enumerate(LEVELS):
        wT = np.zeros((L["C"], RHS_W), dtype=np.float32)
        wT[:, :NA * NO] = (W_SCALE * ws[l][perm]).T
        consts[f"wt{l}"] = wT.astype(f8)
        if has_bias:
            bt = np.zeros((1, RHS_W), dtype=np.float32)
            bt[0, :NA * NO] = (W_SCALE * W_SCALE) * bs[l][perm]
            consts[f"bt{l}"] = bt

    m1 = np.zeros((128, TOT_SLOTS, XY_W), dtype=np.float32)
    g6 = np.zeros((128, TOT_SLOTS, 6), dtype=np.float32)
    for L in LEVELS:
        sb, nsl, nx, stride, S = (L["slot_base"], L["nslots"], L["nx"],
                                  L["stride"], L["S"])
        m1[:, sb:sb + nsl, 0:6] = 2.0 * stride
        anc = np.asarray(L["anchors"], dtype=np.float32).reshape(6)
        m1[:, sb:sb + nsl, 6:12] = 4.0 * anc[None, None, :]
        s = np.arange(nsl * 128)
        valid = s < S
        gx = np.where(valid, (s % nx - 0.5) * stride, 0.0).astype(np.float32)
        gy = np.where(valid, (s // nx - 0.5) * stride, 0.0).astype(np.float32)
        g6[:, sb:sb + nsl, 0::2] = gx.reshape(nsl, 128).T[:, :, None]
        g6[:, sb:sb + nsl, 1::2] = gy.reshape(nsl, 128).T[:, :, None]
    consts["m1"] = np.ascontiguousarray(
        m1.reshape(128, TOT_SLOTS * XY_W)).astype(np.float16)
    consts["g6"] = np.ascontiguousarray(
        g6.reshape(128, TOT_SLOTS * 6)).astype(np.float16)
    return consts


def _make_in_maps(inputs, in_dt="f8", out_dt="f8"):
    import ml_dtypes
    f8 = ml_dtypes.float8_e4m3
    x0 = np.asarray(inputs["x0"], dtype=np.float32)
    x1 = np.asarray(inputs["x1"], dtype=np.float32)
    x2 = np.asarray(inputs["x2"], dtype=np.float32)
    w0 = np.asarray(inputs["w0"], dtype=np.float32)
    w1 = np.asarray(inputs["w1"], dtype=np.float32)
    w2 = np.asarray(inputs["w2"], dtype=np.float32)
    b0 = np.asarray(inputs["b0"], dtype=np.float32)
    b1 = np.asarray(inputs["b1"], dtype=np.float32)
    b2 = np.asarray(inputs["b2"], dtype=np.float32)

    has_bias = bool(np.any(b0) or np.any(b1) or np.any(b2))
    consts = _host_consts(w0, w1, w2, b0, b1, b2, has_bias)

    xr = [(W_SCALE * x0.reshape(B_TOTAL, LEVELS[0]["C"], LEVELS[0]["S"]))
          .astype(f8),
          (W_SCALE * x1.reshape(B_TOTAL, LEVELS[1]["C"], LEVELS[1]["S"]))
          .astype(f8),
          (W_SCALE * x2.reshape(B_TOTAL, LEVELS[2]["C"], LEVELS[2]["S"]))
          .astype(f8)]

    in_maps = []
    for i in range(N_CORES):
        m = dict(consts)
        for l in range(3):
            m[f"x{l}"] = xr[l][B_LOC * i:B_LOC * (i + 1)]
        in_maps.append(m)
    return in_maps, has_bias


def _assemble_core(res, dst):
    """res osc [B,128,67,243] f8 + oxy [B,128,67,12] f16 -> dst [B,25200,85]."""
    oxy = np.asarray(res["oxy"]).astype(np.float32)
    sc = np.asarray(res["osc"]).astype(np.float32)
    for L in LEVELS:
        S, nsl, sb = L["S"], L["nslots"], L["slot_base"]
        xy = oxy[:, :, sb:sb + nsl, :].transpose(0, 2, 1, 3) \
            .reshape(B_LOC, nsl * 128, XY_W)[:, :S]
        s = sc[:, :, sb:sb + nsl, :].transpose(0, 2, 1, 3) \
            .reshape(B_LOC, nsl * 128, SC_W)[:, :S]
        d = dst[:, L["base"]:L["base"] + NA * S].reshape(B_LOC, NA, S, NO)
        for a in range(NA):
            d[:, a, :, 0] = xy[:, :, 2 * a]
            d[:, a, :, 1] = xy[:, :, 2 * a + 1]
            d[:, a, :, 2] = xy[:, :, 6 + 2 * a]
            d[:, a, :, 3] = xy[:, :, 7 + 2 * a]
            d[:, a, :, 4:] = s[:, :, (NO - 4) * a:(NO - 4) * (a + 1)]


def _assemble(results):
    out = np.empty((B_TOTAL, ROWS_PER_B, NO), dtype=np.float32)
    for i in range(N_CORES):
        _assemble_core(results[i], out[B_LOC * i:B_LOC * (i + 1)])
    return out


IN_DT = "f8"
OUT_DT = "f8"


def _run(inputs, trace=False):
    in_maps, has_bias = _make_in_maps(inputs, IN_DT, OUT_DT)
    nc = _get_program(has_bias, in_dt=IN_DT, out_dt=OUT_DT)
    res = run_bass_kernel_spmd(nc, in_maps, core_ids=list(range(N_CORES)),
                               trace=trace)
    return _assemble(res.results), res


def kernel(**inputs):
    out, _ = _run(inputs, trace=False)
    return out


# revision 25
# speedup vs baseline: 1.1909x; 1.0003x over previous
"""YOLOv5 Detect head (conv 1x1 + sigmoid decode) on 8 Trainium2 NeuronCores.

Data-parallel over batch: core i handles batches [2i, 2i+1].

Per (batch, level) the work is h = W @ x  (W [255, C], x [C, ny*nx]) followed
by the YOLO decode.  On device we compute psum[s, o] = sum_c x[c, s] * wT[c, o]
with the *data* as the stationary operand (lhsT = x tile [K=128, M=128 spatial])
and wT [K=128, 256] as the moving operand, so the matmul output lands directly
in [spatial, output-channel] orientation.

Optimizations over the f16 baseline:
  - fp8 (e4m3) x and W, each pre-scaled by 16 on host (avoids subnormal
    flush for the small weights); the sigmoid's free input scale (1/256)
    undoes it.  Halves input DMA bytes; matmul runs at full rate via FWL.
  - W columns permuted: cols 0..11 = xywh channels (x,y per anchor then
    w,h per anchor), cols 12..254 = obj/cls scores, col 255 = zero pad.
  - Scores (95% of output bytes): per-group ACT sigmoid PSUM -> fp8 tile
    -> DMA with per-partition-contiguous G*243B runs.
  - xywh: DVE copies pre-sigmoid psum cols 0:12 (f16) into a per-batch
    tile; one batch-end sigmoid + 3 DVE ops decode all 67 slots at once
    (per-slot constants M1/G6/anchors are baked host-side), one DMA.
"""

import numpy as np
from contextlib import ExitStack

import concourse.bacc as bacc
import concourse.bass as bass
import concourse.mybir as mybir
import concourse.tile as tile
from concourse.bass_utils import run_bass_kernel_spmd

F32 = mybir.dt.float32
F16 = mybir.dt.float16
F8 = mybir.dt.float8e4
AF = mybir.ActivationFunctionType
OP = mybir.AluOpType

NA, NO = 3, 85
B_TOTAL, N_CORES, B_LOC = 16, 8, 2
RHS_W = 256          # 12 xywh + 243 scores + 1 pad
XY_W = 12            # decoded channels per spatial row
SC_W = NA * (NO - 4)  # 243 score channels
GRP = 8              # slots (128 spatial rows each) per psum group (4 banks)
ROWS_PER_B = 25200
W_SCALE = 16.0       # host-side scale on each of x and w; ACT undoes 1/256

LEVELS = [
    dict(C=256, nx=80, ny=80, stride=8.0,
         anchors=((10.0, 13.0), (16.0, 30.0), (33.0, 23.0)), base=0),
    dict(C=512, nx=40, ny=40, stride=16.0,
         anchors=((30.0, 61.0), (62.0, 45.0), (59.0, 119.0)), base=19200),
    dict(C=1024, nx=20, ny=20, stride=32.0,
         anchors=((116.0, 90.0), (156.0, 198.0), (373.0, 326.0)), base=24000),
]
for _L in LEVELS:
    _L["S"] = _L["nx"] * _L["ny"]
    _L["KT"] = _L["C"] // 128
    _L["nslots"] = (_L["S"] + 127) // 128
_SB = 0
for _L in LEVELS:
    _L["slot_base"] = _SB
    _SB += _L["nslots"]
TOT_SLOTS = _SB  # 67


def _perm():
    """perm[new_col] = original output-channel row of W (0..254)."""
    p = np.empty(NA * NO, dtype=np.int64)
    for j in range(6):
        p[j] = (j // 2) * NO + (j % 2)          # x,y per anchor
    for j in range(6):
        p[6 + j] = (j // 2) * NO + 2 + (j % 2)  # w,h per anchor
    for a in range(NA):
        for i in range(NO - 4):
            p[XY_W + a * (NO - 4) + i] = a * NO + 4 + i
    return p


def _groups(S):
    """Yield (slot0, n_slots_in_group, rows_in_last_slot)."""
    full, rem = divmod(S, 128)
    gs = [[t0, min(GRP, full - t0), 128] for t0 in range(0, full, GRP)]
    if rem:
        if gs and gs[-1][1] < GRP:
            gs[-1][1] += 1
            gs[-1][2] = rem
        else:
            gs.append([full, 1, rem])
    return [tuple(g) for g in gs]


def _build_program(has_bias: bool, repeat: int = 1, stages: str = "imavo",
                   in_dt: str = "f8", out_dt: str = "f8", dup: int = 1):
    nc = bacc.Bacc("TRN2", target_bir_lowering=False, debug=False,
                   num_devices=N_CORES)

    xs = [nc.dram_tensor(f"x{l}", [B_LOC, L["C"], L["S"]], F8,
                         kind="ExternalInput") for l, L in enumerate(LEVELS)]
    wts = [nc.dram_tensor(f"wt{l}", [L["C"], RHS_W], F8,
                          kind="ExternalInput") for l, L in enumerate(LEVELS)]
    m1_d = nc.dram_tensor("m1", [128, TOT_SLOTS * XY_W], F16,
                          kind="ExternalInput")
    g6_d = nc.dram_tensor("g6", [128, TOT_SLOTS * 6], F16,
                          kind="ExternalInput")
    if has_bias:
        bts = [nc.dram_tensor(f"bt{l}", [1, RHS_W], F32,
                              kind="ExternalInput") for l, L in enumerate(LEVELS)]
    # outputs: per-partition-contiguous layout [b, p, slot, ch]
    osc = nc.dram_tensor("osc", [B_LOC, 128, TOT_SLOTS, SC_W], F8,
                         kind="ExternalOutput")
    oxy = nc.dram_tensor("oxy", [B_LOC, 128, TOT_SLOTS, XY_W], F16,
                         kind="ExternalOutput")

    with tile.TileContext(nc) as tc, ExitStack() as ctx:
        cpool = ctx.enter_context(tc.tile_pool(name="consts", bufs=1))
        xpools = [ctx.enter_context(tc.tile_pool(name=f"x{l}", bufs=2))
                  for l in range(3)]
        ppool = ctx.enter_context(tc.tile_pool(name="ps", bufs=2, space="PSUM"))
        scpool = ctx.enter_context(tc.tile_pool(name="sc", bufs=2))
        hpool = ctx.enter_context(tc.tile_pool(name="h12", bufs=2))
        dpool = ctx.enter_context(tc.tile_pool(name="dec", bufs=2))
        hspool = ctx.enter_context(tc.tile_pool(name="hs", bufs=2))

        # --- resident constants (ACT hwdge ring: keep the SP ring free for
        # the x planes so the first matmul isn't queued behind ~0.8MB) ---
        wt_tiles, bt_tiles = [], []
        for l, L in enumerate(LEVELS):
            KT = L["KT"]
            wt = cpool.tile([128, KT * RHS_W], F8, tag=f"wt{l}")
            nc.scalar.dma_start(
                wt[:].rearrange("p (k c) -> p k c", c=RHS_W),
                wts[l][:].rearrange("(k p) c -> p k c", p=128))
            wt_tiles.append(wt)
            if has_bias:
                bt = cpool.tile([1, RHS_W], F32, tag=f"bt{l}")
                nc.scalar.dma_start(bt[:], bts[l][:])
                bt_tiles.append(bt)
        m1_t = cpool.tile([128, TOT_SLOTS * XY_W], F16, tag="m1")
        nc.scalar.dma_start(m1_t[:], m1_d[:])
        g6_t = cpool.tile([128, TOT_SLOTS * 6], F16, tag="g6")
        nc.scalar.dma_start(g6_t[:], g6_d[:])
        if has_bias:
            ones = cpool.tile([1, 128], F32, tag="ones")
            nc.vector.memset(ones[:], 1.0)

        inv = 1.0 / (W_SCALE * W_SCALE)

        # --- main loop ---
        def _emit_decode(b, h12, t_lo, t_hi, tag):
            """Sigmoid + decode slots [t_lo, t_hi) of h12, DMA to oxy."""
            n = t_hi - t_lo
            W = n * XY_W
            s12 = dpool.tile([128, TOT_SLOTS * XY_W], F16, tag=f"s12{tag}")
            nc.scalar.activation(s12[:, 0:W],
                                 h12[:, t_lo * XY_W:t_hi * XY_W],
                                 AF.Sigmoid, scale=inv)
            t12 = dpool.tile([128, TOT_SLOTS * XY_W], F16, tag=f"t12{tag}")
            # t = sigma * M1   (M1: 2*stride for xy cols, 4*anchor for wh)
            nc.vector.tensor_tensor(t12[:, 0:W], s12[:, 0:W],
                                    m1_t[:, t_lo * XY_W:t_hi * XY_W], OP.mult)
            s12_v = s12[:].rearrange("p (t j) -> p t j", j=XY_W)
            t12_v = t12[:].rearrange("p (t j) -> p t j", j=XY_W)
            g6_v = g6_t[:].rearrange("p (t j) -> p t j", j=6)
            # xy = t + (grid - 0.5) * stride
            nc.vector.tensor_tensor(s12_v[:, 0:n, 0:6], t12_v[:, 0:n, 0:6],
                                    g6_v[:, t_lo:t_hi, :], OP.add)
            # wh = t * sigma = 4 * anchor * sigma^2
            nc.vector.tensor_tensor(s12_v[:, 0:n, 6:XY_W],
                                    t12_v[:, 0:n, 6:XY_W],
                                    s12_v[:, 0:n, 6:XY_W], OP.mult)
            if "o" in stages:
                nc.scalar.dma_start(oxy[b][:, t_lo:t_hi, :], s12_v[:, 0:n, :])

        def _emit_body():
          for b in range(B_LOC):
            h12 = hpool.tile([128, TOT_SLOTS * XY_W], F16, tag="h12")
            h12_v = h12[:].rearrange("p (t j) -> p t j", j=XY_W)

            xt_vs, sc_tiles, sc_vs = [], [], []
            n_defer = sum(1 for (_, G, _) in _groups(LEVELS[0]["S"])
                          if G == GRP)
            hs_v = None
            for l, L in enumerate(LEVELS):
                KT, S = L["KT"], L["S"]
                Wl = L["nslots"] * 128  # padded spatial width
                x_v = xs[l][b].rearrange("(k p) s -> p k s", p=128)
                # whole-level x tile; one DMA per (b, level) gives S-byte
                # contiguous runs per partition and minimizes HWDGE load
                xt = xpools[l].tile([128, KT * Wl], F8, tag=f"x{l}")
                xt_v = xt[:].rearrange("p (k s) -> p k s", s=Wl)
                xt_vs.append(xt_v)
                if "i" in stages:
                    if b == 0 and l == 0:
                        # head-split: first matmul group only needs the
                        # first GRP slots, don't make it wait for 1.6MB
                        hd = GRP * 128
                        nc.sync.dma_start(xt_v[:, :, 0:hd],
                                          x_v[:, :, 0:hd])
                        nc.sync.dma_start(xt_v[:, :, hd:S],
                                          x_v[:, :, hd:S])
                    else:
                        nc.sync.dma_start(xt_v[:, :, 0:S], x_v[:, :, :])
                    if S < Wl:
                        nc.vector.memset(
                            xt_v[:, :, S:Wl].bitcast(mybir.dt.uint32), 0)
                if "a" in stages:
                    # per-level fp8 score staging; ACT fills it per psum
                    # group, one big DMA per (b, level) writes it out
                    sc = scpool.tile([128, L["nslots"] * SC_W], F8,
                                     tag=f"sc{l}")
                    sc_tiles.append(sc)
                    sc_vs.append(sc[:].rearrange("p (t c) -> p t c", c=SC_W))
                    if l == 0 and n_defer and "v" in stages:
                        hs = hspool.tile([128, n_defer * 2 * SC_W], F16,
                                         tag="hs")
                        hs_v = hs[:].rearrange("p (g s c) -> p g s c",
                                               s=2, c=SC_W)
            if "m" not in stages:
                continue

            # interleave the ACT-heavy L0 groups with PE-heavy L1/L2 groups
            # so the psum drain chain hides under neighbor compute; L2's
            # group stays last (smallest decode tail)
            l0g = [(0, g) for g in _groups(LEVELS[0]["S"])]
            l1g = [(1, g) for g in _groups(LEVELS[1]["S"])]
            l2g = [(2, g) for g in _groups(LEVELS[2]["S"])]
            order, ri = [], 0
            for i, g in enumerate(l0g):
                order.append(g)
                if i % 2 == 1 and ri < len(l1g):
                    order.append(l1g[ri])
                    ri += 1
            order.extend(l1g[ri:])
            order.extend(l2g)
            remaining = [len(_groups(L["S"])) for L in LEVELS]

            for (l, (t0, G, M)) in order:
                L = LEVELS[l]
                KT, sbase = L["KT"], L["slot_base"]
                xt_v = xt_vs[l]
                ps = ppool.tile([128, GRP * RHS_W], F32, tag="ps")
                ps_v = ps[:].rearrange("p (g w) -> p g w", w=RHS_W)
                for j in range(G):
                    po = ps[:, j * RHS_W:(j + 1) * RHS_W]
                    for k in range(KT):
                        nc.tensor.matmul(
                            po,
                            lhsT=xt_v[:, k,
                                      (t0 + j) * 128:(t0 + j + 1) * 128],
                            rhs=wt_tiles[l][:].rearrange(
                                "p (k c) -> p k c", c=RHS_W)[:, k, :],
                            start=(k == 0),
                            stop=(k == KT - 1 and not has_bias))
                    if has_bias:
                        nc.tensor.matmul(po, lhsT=ones[0:1, :],
                                         rhs=bt_tiles[l][0:1, :],
                                         start=False, stop=True)

                # scores: sigmoid psum -> fp8 staging.  For L0's full
                # groups, drain the last 2 slots via DVE (f16 raw h) and
                # sigmoid them in one deferred batched ACT from SBUF so
                # the per-group ACT stays under the PE group time.
                defer = (l == 0 and G == GRP and "a" in stages
                         and "v" in stages)
                if "a" in stages:
                    sc_v = sc_vs[l]
                    nd = G - 2 if defer else G
                    nc.scalar.activation(sc_v[:, t0:t0 + nd, :],
                                         ps_v[:, 0:nd, XY_W:XY_W + SC_W],
                                         AF.Sigmoid, scale=inv)
                    if defer:
                        nc.vector.tensor_copy(
                            hs_v[:, t0 // GRP, :, :],
                            ps_v[:, G - 2:G, XY_W:XY_W + SC_W])

                # xywh: stash pre-sigmoid psum cols 0:12 (f16)
                if "v" in stages:
                    nc.vector.tensor_copy(
                        h12_v[:, sbase + t0:sbase + t0 + G, :],
                        ps_v[:, 0:G, 0:XY_W])

                remaining[l] -= 1
                if remaining[l] == 0 and "a" in stages:
                    if l == 0 and n_defer and "v" in stages:
                        # deferred sigmoid of the DVE-drained L0 slots
                        sc4 = sc_tiles[0][:, 0:n_defer * GRP * SC_W] \
                            .rearrange("p (g s c) -> p g s c",
                                       s=GRP, c=SC_W)
                        nc.scalar.activation(sc4[:, :, GRP - 2:GRP, :],
                                             hs_v[:, 0:n_defer, :, :],
                                             AF.Sigmoid, scale=inv)
                    if "o" in stages:
                        nc.scalar.dma_start(
                            osc[b][:, sbase:sbase + L["nslots"], :],
                            sc_vs[l])
                # decode early once L0+L1 slots are all stashed; L2's 4
                # slots are the only decode left in the kernel tail
                if ("v" in stages and remaining[0] == 0
                        and remaining[1] == 0 and remaining[2] > 0
                        and l != 2):
                    _emit_decode(b, h12, 0, LEVELS[2]["slot_base"], "a")
            if "v" in stages:
                _emit_decode(b, h12, LEVELS[2]["slot_base"], TOT_SLOTS, "b")

        if repeat == 1:
            _emit_body()
        else:
            # timing-only mode: run the same body `repeat` times via a
            # hardware loop (program size stays constant)
            with tc.For_i(0, repeat, 1,
                          hint_engines=(mybir.EngineType.PE,)):
                for _ in range(dup):
                    _emit_body()

    nc.compile()
    return nc


_PROG_CACHE = {}


def _get_program(has_bias: bool, repeat: int = 1, stages: str = "imavo",
                 in_dt: str = "f8", out_dt: str = "f8", dup: int = 1):
    key = (has_bias, repeat, stages, in_dt, out_dt, dup)
    if key not in _PROG_CACHE:
        _PROG_CACHE[key] = _build_program(has_bias, repeat, stages, in_dt,
                                          out_dt, dup)
    return _PROG_CACHE[key]


def _host_consts(w0, w1, w2, b0, b1, b2, has_bias):
    """Precompute replicated constant arrays shared by all cores."""
    import ml_dtypes
    f8 = ml_dtypes.float8_e4m3
    perm = _perm()
    consts = {}
    ws, bs = (w0, w1, w2), (b0, b1, b2)
    for l, L in enumerate(LEVELS):
        wT = np.zeros((L["C"], RHS_W), dtype=np.float32)
        wT[:, :NA * NO] = (W_SCALE * ws[l][perm]).T
        consts[f"wt{l}"] = wT.astype(f8)
        if has_bias:
            bt = np.zeros((1, RHS_W), dtype=np.float32)
            bt[0, :NA * NO] = (W_SCALE * W_SCALE) * bs[l][perm]
            consts[f"bt{l}"] = bt

    m1 = np.zeros((128, TOT_SLOTS, XY_W), dtype=np.float32)
    g6 = np.zeros((128, TOT_SLOTS, 6), dtype=np.float32)
    for L in LEVELS:
        sb, nsl, nx, stride, S = (L["slot_base"], L["nslots"], L["nx"],
                                  L["stride"], L["S"])
        m1[:, sb:sb + nsl, 0:6] = 2.0 * stride
        anc = np.asarray(L["anchors"], dtype=np.float32).reshape(6)
        m1[:, sb:sb + nsl, 6:12] = 4.0 * anc[None, None, :]
        s = np.arange(nsl * 128)
        valid = s < S
        gx = np.where(valid, (s % nx - 0.5) * stride, 0.0).astype(np.float32)
        gy = np.where(valid, (s // nx - 0.5) * stride, 0.0).astype(np.float32)
        g6[:, sb:sb + nsl, 0::2] = gx.reshape(nsl, 128).T[:, :, None]
        g6[:, sb:sb + nsl, 1::2] = gy.reshape(nsl, 128).T[:, :, None]
    consts["m1"] = np.ascontiguousarray(
        m1.reshape(128, TOT_SLOTS * XY_W)).astype(np.float16)
    consts["g6"] = np.ascontiguousarray(
        g6.reshape(128, TOT_SLOTS * 6)).astype(np.float16)
    return consts


def _make_in_maps(inputs, in_dt="f8", out_dt="f8"):
    import ml_dtypes
    f8 = ml_dtypes.float8_e4m3
    x0 = np.asarray(inputs["x0"], dtype=np.float32)
    x1 = np.asarray(inputs["x1"], dtype=np.float32)
    x2 = np.asarray(inputs["x2"], dtype=np.float32)
    w0 = np.asarray(inputs["w0"], dtype=np.float32)
    w1 = np.asarray(inputs["w1"], dtype=np.float32)
    w2 = np.asarray(inputs["w2"], dtype=np.float32)
    b0 = np.asarray(inputs["b0"], dtype=np.float32)
    b1 = np.asarray(inputs["b1"], dtype=np.float32)
    b2 = np.asarray(inputs["b2"], dtype=np.float32)

    has_bias = bool(np.any(b0) or np.any(b1) or np.any(b2))
    consts = _host_consts(w0, w1, w2, b0, b1, b2, has_bias)

    xr = [(W_SCALE * x0.reshape(B_TOTAL, LEVELS[0]["C"], LEVELS[0]["S"]))
          .astype(f8),
          (W_SCALE * x1.reshape(B_TOTAL, LEVELS[1]["C"], LEVELS[1]["S"]))
          .astype(f8),
          (W_SCALE * x2.reshape(B_TOTAL, LEVELS[2]["C"], LEVELS[2]["S"]))
          .astype(f8)]

    in_maps = []
    for i in range(N_CORES):
        m = dict(consts)
        for l in range(3):
            m[f"x{l}"] = xr[l][B_LOC * i:B_LOC * (i + 1)]
        in_maps.append(m)
    return in_maps, has_bias


def _assemble_core(res, dst):
    """res osc [B,128,67,243] f8 + oxy [B,128,67,12] f16 -> dst [B,25200,85]."""
    oxy = np.asarray(res["oxy"]).astype(np.float32)
    sc = np.asarray(res["osc"]).astype(np.float32)
    for L in LEVELS:
        S, nsl, sb = L["S"], L["nslots"], L["slot_base"]
        xy = oxy[:, :, sb:sb + nsl, :].transpose(0, 2, 1, 3) \
            .reshape(B_LOC, nsl * 128, XY_W)[:, :S]
        s = sc[:, :, sb:sb + nsl, :].transpose(0, 2, 1, 3) \
            .reshape(B_LOC, nsl * 128, SC_W)[:, :S]
        d = dst[:, L["base"]:L["base"] + NA * S].reshape(B_LOC, NA, S, NO)
        for a in range(NA):
            d[:, a, :, 0] = xy[:, :, 2 * a]
            d[:, a, :, 1] = xy[:, :, 2 * a + 1]
            d[:, a, :, 2] = xy[:, :, 6 + 2 * a]
            d[:, a, :, 3] = xy[:, :, 7 + 2 * a]
            d[:, a, :, 4:] = s[:, :, (NO - 4) * a:(NO - 4) * (a + 1)]


def _assemble(results):
    out = np.empty((B_TOTAL, ROWS_PER_B, NO), dtype=np.float32)
    for i in range(N_CORES):
        _assemble_core(results[i], out[B_LOC * i:B_LOC * (i + 1)])
    return out


IN_DT = "f8"
OUT_DT = "f8"


def _run(inputs, trace=False):
    in_maps, has_bias = _make_in_maps(inputs, IN_DT, OUT_DT)
    nc = _get_program(has_bias, in_dt=IN_DT, out_dt=OUT_DT)
    res = run_bass_kernel_spmd(nc, in_maps, core_ids=list(range(N_CORES)),
                               trace=trace)
    return _assemble(res.results), res


def kernel(**inputs):
    out, _ = _run(inputs, trace=False)
    return out


# revision 27
# speedup vs baseline: 1.1987x; 1.0066x over previous
"""YOLOv5 Detect head (conv 1x1 + sigmoid decode) on 8 Trainium2 NeuronCores.

Data-parallel over batch: core i handles batches [2i, 2i+1].

Per (batch, level) the work is h = W @ x  (W [255, C], x [C, ny*nx]) followed
by the YOLO decode.  On device we compute psum[s, o] = sum_c x[c, s] * wT[c, o]
with the *data* as the stationary operand (lhsT = x tile [K=128, M=128 spatial])
and wT [K=128, 256] as the moving operand, so the matmul output lands directly
in [spatial, output-channel] orientation.

Optimizations over the f16 baseline:
  - fp8 (e4m3) x and W, each pre-scaled by 16 on host (avoids subnormal
    flush for the small weights); the sigmoid's free input scale (1/256)
    undoes it.  Halves input DMA bytes; matmul runs at full rate via FWL.
  - W columns permuted: cols 0..11 = xywh channels (x,y per anchor then
    w,h per anchor), cols 12..254 = obj/cls scores, col 255 = zero pad.
  - Scores (95% of output bytes): per-group ACT sigmoid PSUM -> fp8 tile
    -> DMA with per-partition-contiguous G*243B runs.
  - xywh: DVE copies pre-sigmoid psum cols 0:12 (f16) into a per-batch
    tile; one batch-end sigmoid + 3 DVE ops decode all 67 slots at once
    (per-slot constants M1/G6/anchors are baked host-side), one DMA.
"""

import numpy as np
from contextlib import ExitStack

import concourse.bacc as bacc
import concourse.bass as bass
import concourse.mybir as mybir
import concourse.tile as tile
from concourse.bass_utils import run_bass_kernel_spmd

F32 = mybir.dt.float32
F16 = mybir.dt.float16
F8 = mybir.dt.float8e4
AF = mybir.ActivationFunctionType
OP = mybir.AluOpType

NA, NO = 3, 85
B_TOTAL, N_CORES, B_LOC = 16, 8, 2
RHS_W = 256          # 12 xywh + 243 scores + 1 pad
XY_W = 12            # decoded channels per spatial row
SC_W = NA * (NO - 4)  # 243 score channels
GRP = 8              # slots (128 spatial rows each) per psum group (4 banks)
ROWS_PER_B = 25200
W_SCALE = 16.0       # host-side scale on each of x and w; ACT undoes 1/256

LEVELS = [
    dict(C=256, nx=80, ny=80, stride=8.0,
         anchors=((10.0, 13.0), (16.0, 30.0), (33.0, 23.0)), base=0),
    dict(C=512, nx=40, ny=40, stride=16.0,
         anchors=((30.0, 61.0), (62.0, 45.0), (59.0, 119.0)), base=19200),
    dict(C=1024, nx=20, ny=20, stride=32.0,
         anchors=((116.0, 90.0), (156.0, 198.0), (373.0, 326.0)), base=24000),
]
for _L in LEVELS:
    _L["S"] = _L["nx"] * _L["ny"]
    _L["KT"] = _L["C"] // 128
    _L["nslots"] = (_L["S"] + 127) // 128
_SB = 0
for _L in LEVELS:
    _L["slot_base"] = _SB
    _SB += _L["nslots"]
TOT_SLOTS = _SB  # 67


def _perm():
    """perm[new_col] = original output-channel row of W (0..254)."""
    p = np.empty(NA * NO, dtype=np.int64)
    for j in range(6):
        p[j] = (j // 2) * NO + (j % 2)          # x,y per anchor
    for j in range(6):
        p[6 + j] = (j // 2) * NO + 2 + (j % 2)  # w,h per anchor
    for a in range(NA):
        for i in range(NO - 4):
            p[XY_W + a * (NO - 4) + i] = a * NO + 4 + i
    return p


def _groups(S):
    """Yield (slot0, n_slots_in_group, rows_in_last_slot)."""
    full, rem = divmod(S, 128)
    gs = [[t0, min(GRP, full - t0), 128] for t0 in range(0, full, GRP)]
    if rem:
        if gs and gs[-1][1] < GRP:
            gs[-1][1] += 1
            gs[-1][2] = rem
        else:
            gs.append([full, 1, rem])
    return [tuple(g) for g in gs]


def _build_program(has_bias: bool, repeat: int = 1, stages: str = "imavo",
                   in_dt: str = "f8", out_dt: str = "f8", dup: int = 1):
    nc = bacc.Bacc("TRN2", target_bir_lowering=False, debug=False,
                   num_devices=N_CORES)

    xs = [nc.dram_tensor(f"x{l}", [B_LOC, L["C"], L["S"]], F8,
                         kind="ExternalInput") for l, L in enumerate(LEVELS)]
    wts = [nc.dram_tensor(f"wt{l}", [L["C"], RHS_W], F8,
                          kind="ExternalInput") for l, L in enumerate(LEVELS)]
    m1_d = nc.dram_tensor("m1", [128, TOT_SLOTS * XY_W], F16,
                          kind="ExternalInput")
    g6_d = nc.dram_tensor("g6", [128, TOT_SLOTS * 6], F16,
                          kind="ExternalInput")
    if has_bias:
        bts = [nc.dram_tensor(f"bt{l}", [1, RHS_W], F32,
                              kind="ExternalInput") for l, L in enumerate(LEVELS)]
    # outputs: per-partition-contiguous layout [b, p, slot, ch]
    osc = nc.dram_tensor("osc", [B_LOC, 128, TOT_SLOTS, SC_W], F8,
                         kind="ExternalOutput")
    oxy = nc.dram_tensor("oxy", [B_LOC, 128, TOT_SLOTS, XY_W], F16,
                         kind="ExternalOutput")

    with tile.TileContext(nc) as tc, ExitStack() as ctx:
        cpool = ctx.enter_context(tc.tile_pool(name="consts", bufs=1))
        xpools = [ctx.enter_context(tc.tile_pool(name=f"x{l}", bufs=2))
                  for l in range(3)]
        ppool = ctx.enter_context(tc.tile_pool(name="ps", bufs=2, space="PSUM"))
        scpool = ctx.enter_context(tc.tile_pool(name="sc", bufs=2))
        hpool = ctx.enter_context(tc.tile_pool(name="h12", bufs=2))
        dpool = ctx.enter_context(tc.tile_pool(name="dec", bufs=2))
        hspool = ctx.enter_context(tc.tile_pool(name="hs", bufs=2))

        # --- resident constants (ACT hwdge ring: keep the SP ring free for
        # the x planes so the first matmul isn't queued behind ~0.8MB) ---
        wt_tiles, bt_tiles = [], []
        for l, L in enumerate(LEVELS):
            KT = L["KT"]
            wt = cpool.tile([128, KT * RHS_W], F8, tag=f"wt{l}")
            nc.scalar.dma_start(
                wt[:].rearrange("p (k c) -> p k c", c=RHS_W),
                wts[l][:].rearrange("(k p) c -> p k c", p=128))
            wt_tiles.append(wt)
            if has_bias:
                bt = cpool.tile([1, RHS_W], F32, tag=f"bt{l}")
                nc.scalar.dma_start(bt[:], bts[l][:])
                bt_tiles.append(bt)
        m1_t = cpool.tile([128, TOT_SLOTS * XY_W], F16, tag="m1")
        nc.scalar.dma_start(m1_t[:], m1_d[:])
        g6_t = cpool.tile([128, TOT_SLOTS * 6], F16, tag="g6")
        nc.scalar.dma_start(g6_t[:], g6_d[:])
        if has_bias:
            ones = cpool.tile([1, 128], F32, tag="ones")
            nc.vector.memset(ones[:], 1.0)

        inv = 1.0 / (W_SCALE * W_SCALE)

        # --- main loop ---
        def _emit_decode(b, h12, t_lo, t_hi, tag):
            """Sigmoid + decode slots [t_lo, t_hi) of h12, DMA to oxy."""
            n = t_hi - t_lo
            W = n * XY_W
            s12 = dpool.tile([128, TOT_SLOTS * XY_W], F16, tag=f"s12{tag}")
            nc.scalar.activation(s12[:, 0:W],
                                 h12[:, t_lo * XY_W:t_hi * XY_W],
                                 AF.Sigmoid, scale=inv)
            t12 = dpool.tile([128, TOT_SLOTS * XY_W], F16, tag=f"t12{tag}")
            # t = sigma * M1   (M1: 2*stride for xy cols, 4*anchor for wh)
            nc.vector.tensor_tensor(t12[:, 0:W], s12[:, 0:W],
                                    m1_t[:, t_lo * XY_W:t_hi * XY_W], OP.mult)
            s12_v = s12[:].rearrange("p (t j) -> p t j", j=XY_W)
            t12_v = t12[:].rearrange("p (t j) -> p t j", j=XY_W)
            g6_v = g6_t[:].rearrange("p (t j) -> p t j", j=6)
            # xy = t + (grid - 0.5) * stride
            nc.vector.tensor_tensor(s12_v[:, 0:n, 0:6], t12_v[:, 0:n, 0:6],
                                    g6_v[:, t_lo:t_hi, :], OP.add)
            # wh = t * sigma = 4 * anchor * sigma^2
            nc.vector.tensor_tensor(s12_v[:, 0:n, 6:XY_W],
                                    t12_v[:, 0:n, 6:XY_W],
                                    s12_v[:, 0:n, 6:XY_W], OP.mult)
            if "o" in stages:
                nc.scalar.dma_start(oxy[b][:, t_lo:t_hi, :], s12_v[:, 0:n, :])

        def _emit_body():
          for b in range(B_LOC):
            h12 = hpool.tile([128, TOT_SLOTS * XY_W], F16, tag="h12")
            h12_v = h12[:].rearrange("p (t j) -> p t j", j=XY_W)

            xt_vs, sc_tiles, sc_vs = [], [], []
            n_defer = sum(1 for (_, G, _) in _groups(LEVELS[0]["S"])
                          if G == GRP)
            hs_v = None
            for l, L in enumerate(LEVELS):
                KT, S = L["KT"], L["S"]
                Wl = L["nslots"] * 128  # padded spatial width
                x_v = xs[l][b].rearrange("(k p) s -> p k s", p=128)
                # whole-level x tile; one DMA per (b, level) gives S-byte
                # contiguous runs per partition and minimizes HWDGE load
                xt = xpools[l].tile([128, KT * Wl], F8, tag=f"x{l}")
                xt_v = xt[:].rearrange("p (k s) -> p k s", s=Wl)
                xt_vs.append(xt_v)
                if "i" in stages:
                    if b == 0 and l == 0:
                        # head-split: first matmul group only needs the
                        # first GRP slots, don't make it wait for 1.6MB
                        hd = GRP * 128
                        nc.sync.dma_start(xt_v[:, :, 0:hd],
                                          x_v[:, :, 0:hd])
                        nc.sync.dma_start(xt_v[:, :, hd:S],
                                          x_v[:, :, hd:S])
                    else:
                        nc.sync.dma_start(xt_v[:, :, 0:S], x_v[:, :, :])
                    if S < Wl:
                        nc.vector.memset(
                            xt_v[:, :, S:Wl].bitcast(mybir.dt.uint32), 0)
                if "a" in stages:
                    # per-level fp8 score staging; ACT fills it per psum
                    # group, one big DMA per (b, level) writes it out
                    sc = scpool.tile([128, L["nslots"] * SC_W], F8,
                                     tag=f"sc{l}")
                    sc_tiles.append(sc)
                    sc_vs.append(sc[:].rearrange("p (t c) -> p t c", c=SC_W))
                    if l == 0 and n_defer and "v" in stages:
                        hs = hspool.tile([128, n_defer * 2 * SC_W], F16,
                                         tag="hs")
                        hs_v = hs[:].rearrange("p (g s c) -> p g s c",
                                               s=2, c=SC_W)
            if "m" not in stages:
                continue

            # interleave the ACT-heavy L0 groups with PE-heavy L1/L2 groups
            # so the psum drain chain hides under neighbor compute; L2's
            # group stays last (smallest decode tail)
            l0g = [(0, g) for g in _groups(LEVELS[0]["S"])]
            l1g = [(1, g) for g in _groups(LEVELS[1]["S"])]
            l2g = [(2, g) for g in _groups(LEVELS[2]["S"])]
            order, ri = [], 0
            for i, g in enumerate(l0g):
                order.append(g)
                if i % 2 == 1 and ri < len(l1g):
                    order.append(l1g[ri])
                    ri += 1
            order.extend(l1g[ri:])
            order.extend(l2g)
            remaining = [len(_groups(L["S"])) for L in LEVELS]

            for (l, (t0, G, M)) in order:
                L = LEVELS[l]
                KT, sbase = L["KT"], L["slot_base"]
                xt_v = xt_vs[l]
                ps = ppool.tile([128, GRP * RHS_W], F32, tag="ps")
                ps_v = ps[:].rearrange("p (g w) -> p g w", w=RHS_W)
                for j in range(G):
                    po = ps[:, j * RHS_W:(j + 1) * RHS_W]
                    for k in range(KT):
                        nc.tensor.matmul(
                            po,
                            lhsT=xt_v[:, k,
                                      (t0 + j) * 128:(t0 + j + 1) * 128],
                            rhs=wt_tiles[l][:].rearrange(
                                "p (k c) -> p k c", c=RHS_W)[:, k, :],
                            start=(k == 0),
                            stop=(k == KT - 1 and not has_bias))
                    if has_bias:
                        nc.tensor.matmul(po, lhsT=ones[0:1, :],
                                         rhs=bt_tiles[l][0:1, :],
                                         start=False, stop=True)

                # scores: sigmoid psum -> fp8 staging.  For L0's full
                # groups, drain the last 2 slots via DVE (f16 raw h) and
                # sigmoid them in one deferred batched ACT from SBUF so
                # the per-group ACT stays under the PE group time.
                defer = (l == 0 and G == GRP and "a" in stages
                         and "v" in stages)
                if "a" in stages:
                    sc_v = sc_vs[l]
                    nd = G - 2 if defer else G
                    nc.scalar.activation(sc_v[:, t0:t0 + nd, :],
                                         ps_v[:, 0:nd, XY_W:XY_W + SC_W],
                                         AF.Sigmoid, scale=inv)
                    if defer:
                        nc.vector.tensor_copy(
                            hs_v[:, t0 // GRP, :, :],
                            ps_v[:, G - 2:G, XY_W:XY_W + SC_W])

                # xywh: stash pre-sigmoid psum cols 0:12 (f16)
                if "v" in stages:
                    nc.vector.tensor_copy(
                        h12_v[:, sbase + t0:sbase + t0 + G, :],
                        ps_v[:, 0:G, 0:XY_W])

                remaining[l] -= 1
                if remaining[l] == 0 and "a" in stages:
                    if l == 0 and n_defer and "v" in stages:
                        # deferred sigmoid of the DVE-drained L0 slots
                        sc4 = sc_tiles[0][:, 0:n_defer * GRP * SC_W] \
                            .rearrange("p (g s c) -> p g s c",
                                       s=GRP, c=SC_W)
                        nc.scalar.activation(sc4[:, :, GRP - 2:GRP, :],
                                             hs_v[:, 0:n_defer, :, :],
                                             AF.Sigmoid, scale=inv)
                    if "o" in stages:
                        nc.scalar.dma_start(
                            osc[b][:, sbase:sbase + L["nslots"], :],
                            sc_vs[l])
                # decode early once L0+L1 slots are all stashed; L2's 4
                # slots are the only decode left in the kernel tail
                if ("v" in stages and remaining[0] == 0
                        and remaining[1] == 0 and remaining[2] > 0
                        and l != 2):
                    _emit_decode(b, h12, 0, LEVELS[2]["slot_base"], "a")
            if "v" in stages:
                _emit_decode(b, h12, LEVELS[2]["slot_base"], TOT_SLOTS, "b")

        if repeat == 1:
            _emit_body()
        else:
            # timing-only mode: run the same body `repeat` times via a
            # hardware loop (program size stays constant)
            with tc.For_i(0, repeat, 1,
                          hint_engines=(mybir.EngineType.PE,)):
                for _ in range(dup):
                    _emit_body()

    nc.compile()
    return nc


_PROG_CACHE = {}


def _get_program(has_bias: bool, repeat: int = 1, stages: str = "imavo",
                 in_dt: str = "f8", out_dt: str = "f8", dup: int = 1):
    key = (has_bias, repeat, stages, in_dt, out_dt, dup)
    if key not in _PROG_CACHE:
        _PROG_CACHE[key] = _build_program(has_bias, repeat, stages, in_dt,
                                          out_dt, dup)
    return _PROG_CACHE[key]


def _host_consts(w0, w1, w2, b0, b1, b2, has_bias):
    """Precompute replicated constant arrays shared by all cores."""
    import ml_dtypes
    f8 = ml_dtypes.float8_e4m3
    perm = _perm()
    consts = {}
    ws, bs = (w0, w1, w2), (b0, b1, b2)
    for l, L in enumerate(LEVELS):
        wT = np.zeros((L["C"], RHS_W), dtype=np.float32)
        wT[:, :NA * NO] = (W_SCALE * ws[l][perm]).T
        consts[f"wt{l}"] = wT.astype(f8)
        if has_bias:
            bt = np.zeros((1, RHS_W), dtype=np.float32)
            bt[0, :NA * NO] = (W_SCALE * W_SCALE) * bs[l][perm]
            consts[f"bt{l}"] = bt

    m1 = np.zeros((128, TOT_SLOTS, XY_W), dtype=np.float32)
    g6 = np.zeros((128, TOT_SLOTS, 6), dtype=np.float32)
    for L in LEVELS:
        sb, nsl, nx, stride, S = (L["slot_base"], L["nslots"], L["nx"],
                                  L["stride"], L["S"])
        m1[:, sb:sb + nsl, 0:6] = 2.0 * stride
        anc = np.asarray(L["anchors"], dtype=np.float32).reshape(6)
        m1[:, sb:sb + nsl, 6:12] = 4.0 * anc[None, None, :]
        s = np.arange(nsl * 128)
        valid = s < S
        gx = np.where(valid, (s % nx - 0.5) * stride, 0.0).astype(np.float32)
        gy = np.where(valid, (s // nx - 0.5) * stride, 0.0).astype(np.float32)
        g6[:, sb:sb + nsl, 0::2] = gx.reshape(nsl, 128).T[:, :, None]
        g6[:, sb:sb + nsl, 1::2] = gy.reshape(nsl, 128).T[:, :, None]
    consts["m1"] = np.ascontiguousarray(
        m1.reshape(128, TOT_SLOTS * XY_W)).astype(np.float16)
    consts["g6"] = np.ascontiguousarray(
        g6.reshape(128, TOT_SLOTS * 6)).astype(np.float16)
    return consts


def _make_in_maps(inputs, in_dt="f8", out_dt="f8"):
    import ml_dtypes
    f8 = ml_dtypes.float8_e4m3
    x0 = np.asarray(inputs["x0"], dtype=np.float32)
    x1 = np.asarray(inputs["x1"], dtype=np.float32)
    x2 = np.asarray(inputs["x2"], dtype=np.float32)
    w0 = np.asarray(inputs["w0"], dtype=np.float32)
    w1 = np.asarray(inputs["w1"], dtype=np.float32)
    w2 = np.asarray(inputs["w2"], dtype=np.float32)
    b0 = np.asarray(inputs["b0"], dtype=np.float32)
    b1 = np.asarray(inputs["b1"], dtype=np.float32)
    b2 = np.asarray(inputs["b2"], dtype=np.float32)

    has_bias = bool(np.any(b0) or np.any(b1) or np.any(b2))
    consts = _host_consts(w0, w1, w2, b0, b1, b2, has_bias)

    xr = [(W_SCALE * x0.reshape(B_TOTAL, LEVELS[0]["C"], LEVELS[0]["S"]))
          .astype(f8),
          (W_SCALE * x1.reshape(B_TOTAL, LEVELS[1]["C"], LEVELS[1]["S"]))
          .astype(f8),
          (W_SCALE * x2.reshape(B_TOTAL, LEVELS[2]["C"], LEVELS[2]["S"]))
          .astype(f8)]

    in_maps = []
    for i in range(N_CORES):
        m = dict(consts)
        for l in range(3):
            m[f"x{l}"] = xr[l][B_LOC * i:B_LOC * (i + 1)]
        in_maps.append(m)
    return in_maps, has_bias


def _assemble_core(res, dst):
    """res osc [B,128,67,243] f8 + oxy [B,128,67,12] f16 -> dst [B,25200,85]."""
    oxy = np.asarray(res["oxy"]).astype(np.float32)
    sc = np.asarray(res["osc"]).astype(np.float32)
    for L in LEVELS:
        S, nsl, sb = L["S"], L["nslots"], L["slot_base"]
        xy = oxy[:, :, sb:sb + nsl, :].transpose(0, 2, 1, 3) \
            .reshape(B_LOC, nsl * 128, XY_W)[:, :S]
        s = sc[:, :, sb:sb + nsl, :].transpose(0, 2, 1, 3) \
            .reshape(B_LOC, nsl * 128, SC_W)[:, :S]
        d = dst[:, L["base"]:L["base"] + NA * S].reshape(B_LOC, NA, S, NO)
        for a in range(NA):
            d[:, a, :, 0] = xy[:, :, 2 * a]
            d[:, a, :, 1] = xy[:, :, 2 * a + 1]
            d[:, a, :, 2] = xy[:, :, 6 + 2 * a]
            d[:, a, :, 3] = xy[:, :, 7 + 2 * a]
            d[:, a, :, 4:] = s[:, :, (NO - 4) * a:(NO - 4) * (a + 1)]


def _assemble(results):
    out = np.empty((B_TOTAL, ROWS_PER_B, NO), dtype=np.float32)
    for i in range(N_CORES):
        _assemble_core(results[i], out[B_LOC * i:B_LOC * (i + 1)])
    return out


IN_DT = "f8"
OUT_DT = "f8"


def _run(inputs, trace=False):
    in_maps, has_bias = _make_in_maps(inputs, IN_DT, OUT_DT)
    nc = _get_program(has_bias, in_dt=IN_DT, out_dt=OUT_DT)
    res = run_bass_kernel_spmd(nc, in_maps, core_ids=list(range(N_CORES)),
                               trace=trace)
    return _assemble(res.results), res


def kernel(**inputs):
    out, _ = _run(inputs, trace=False)
    return out
